# revision 2
# baseline (speedup 1.0000x reference)
"""Deformable KPConv layer on 8 Trainium2 NeuronCores (Bass/Tile).

Strategy (data-parallel over the 16384 query points, 2048/core):
  - features are pre-cast to bf16 host-side and gathered per-edge from HBM
    via multi-index indirect DMA into an "edge-slot" layout
    [(4 queries x 32 neighbors) partitions, group, 128 feat].
  - support coords (+|s|^2) gathered per-edge in query-partition layout.
  - squared distances to (possibly deformed) kernel points are computed on
    DVE/GpSimd as  |s|^2 + |C|^2 - 2 s.C  with C = q + kp (+ offset),
    influence w' = relu(2 - d) on ScalarE (the 1/2 is folded into the
    conv weights host-side).
  - the neighbor contraction runs on TensorE as block-diagonal matmuls:
    w' is scattered into a zero-initialized block-diagonal SBUF tile by
    4 strided SBUF->SBUF DMAs, then  psum[f,(q,k)] = nf^T @ wblk.
  - the (k,f)->42 offset projection and (k,f)->256 output projection are
    PSUM-accumulated matmuls with the drained wf tiles as stationary
    operands, producing query-partition outputs directly.
"""

import os
import sys

sys.path.insert(0, "/opt/trn_rl_repo")

import numpy as np
import ml_dtypes

import concourse.bass as bass
import concourse.tile as tile
from concourse import bacc, mybir

N_Q = 16384
N_S = 16384
NN = 32
F_IN = 128
F_OUT = 256
K = 15
DIM = 3
OFF_DIM = DIM * (K - 1)  # 42
EXTENT = 2.0
N_CORES = 8
P = 128

BF16 = mybir.dt.bfloat16
F32 = mybir.dt.float32
I32 = mybir.dt.int32


def build_nc(qpc: int, reps: int = 1):
    """Build the single-core SPMD Bass program for qpc queries per core."""
    T = qpc // P  # query tiles per core
    NG = P // 4  # 32 groups of 4 queries per tile

    nc = bacc.Bacc("TRN2", target_bir_lowering=False)

    nfg_d = nc.dram_tensor("nfg", [T, P, NN, F_IN], BF16, kind="ExternalInput")
    spg_d = nc.dram_tensor("spg", [T, P, NN, 4], F32, kind="ExternalInput")
    qc_d = nc.dram_tensor("qc", [T, P, 4], F32, kind="ExternalInput")
    kprep_d = nc.dram_tensor("kprep", [P, K * DIM], F32, kind="ExternalInput")
    dwsb_d = nc.dram_tensor("dwsb", [P, K * OFF_DIM], BF16, kind="ExternalInput")
    wsb_d = nc.dram_tensor("wsb", [P, K * F_OUT], BF16, kind="ExternalInput")
    brep_d = nc.dram_tensor("brep", [P, OFF_DIM], F32, kind="ExternalInput")
    out_d = nc.dram_tensor("out", [qpc, F_OUT], F32, kind="ExternalOutput")

    NK = NN * K  # 480
    lvl = int(os.environ.get("KLVL", "5"))
    skips = set(os.environ.get("KSKIP", "").split(","))

    with tile.TileContext(nc) as tc:
        with (
            tc.tile_pool(name="const", bufs=1) as cpool,
            tc.tile_pool(name="persist", bufs=1) as ppool,
            tc.tile_pool(name="idx", bufs=2) as ipool,
            tc.tile_pool(name="nf", bufs=3) as nfpool,
            tc.tile_pool(name="sp", bufs=3) as sppool,
            tc.tile_pool(name="sq", bufs=3) as sqpool,
            tc.tile_pool(name="wd", bufs=3) as wdpool,
            tc.tile_pool(name="wf", bufs=2) as wfpool,
            tc.tile_pool(name="cc", bufs=2) as ccpool,
            tc.tile_pool(name="outp", bufs=2) as opool,
            tc.tile_pool(name="dram", bufs=4, space="DRAM") as drpool,
            tc.tile_pool(name="ps", bufs=3, space="PSUM") as pspool,
            tc.tile_pool(name="ps2", bufs=2, space="PSUM") as ps2pool,
        ):
            # --- constants, loaded once ---
            kprep = cpool.tile([P, K, DIM], F32, tag="kprep")
            nc.sync.dma_start(out=kprep[:], in_=kprep_d[:].rearrange("p (k d) -> p k d", d=DIM))
            dwsb = cpool.tile([P, K * OFF_DIM], BF16, tag="dwsb")
            nc.sync.dma_start(out=dwsb[:], in_=dwsb_d[:])
            wsb = cpool.tile([P, K * F_OUT], BF16, tag="wsb")
            nc.sync.dma_start(out=wsb[:], in_=wsb_d[:])
            brep = cpool.tile([P, OFF_DIM], F32, tag="brep")
            nc.sync.dma_start(out=brep[:], in_=brep_d[:])
            two_c = cpool.tile([P, 1], F32, tag="two")
            nc.vector.memset(two_c[:], 2.0)
            eps_c = cpool.tile([P, 1], F32, tag="eps")
            nc.vector.memset(eps_c[:], 1e-5)

            # persistent block-diagonal tiles (zeros off-diagonal, never touched
            # there again: the scatter DMAs only overwrite the diagonal blocks)
            wblks = []
            for i in range(6):
                wb = nc.alloc_sbuf_tensor(f"wblk{i}", [P, NG, 4 * K], BF16)
                nc.gpsimd.memset(wb.ap(), 0.0)
                wblks.append(wb)

            import itertools
            for _rep, t in itertools.product(range(reps), range(T)):
                # --- loads (pregathered on host per sharding strategy) ---
                qc_t = ipool.tile([P, 4], F32, tag="qc")
                nc.sync.dma_start(out=qc_t[:], in_=qc_d[t])
                nf = nfpool.tile([P, NN, F_IN], BF16, tag="nf")
                nc.sync.dma_start(out=nf[:], in_=nfg_d[t])
                sp = sppool.tile([P, NN, 4], F32, tag="sp")
                nc.sync.dma_start(out=sp[:], in_=spg_d[t])

                if lvl < 2:
                    out_sb0 = opool.tile([P, F_OUT], F32, tag="outsb")
                    nc.vector.memset(out_sb0[:], 0.0)
                    nc.vector.tensor_copy(out=out_sb0[:, 0:NN], in_=sp[:, :, 3])
                    nc.vector.tensor_copy(out=out_sb0[:, NN:NN+NN], in_=nf[:, 0, 0:NN])
                    nc.sync.dma_start(out=out_d[t * P : (t + 1) * P, :], in_=out_sb0[:])
                    continue
                # s' = -2*s, stored d-major [P, 4, NN]; row 3 = +|s|^2
                sp4t = sppool.tile([P, 4, NN], F32, tag="sp4t")
                nc.vector.tensor_scalar(
                    out=sp4t[:, 0:DIM, :],
                    in0=sp[:].transpose([0, 2, 1])[:, 0:DIM, :],
                    scalar1=-2.0,
                    scalar2=None,
                    op0=mybir.AluOpType.mult,
                )
                nc.vector.tensor_scalar(
                    out=sp4t[:, 3, :],
                    in0=sp[:, :, 3],
                    scalar1=1.0,
                    scalar2=None,
                    op0=mybir.AluOpType.mult,
                )

                # C0[q, k, d] = q_d + kp[k, d]
                c0 = ccpool.tile([P, K, DIM], F32, tag="c0")
                nc.vector.tensor_tensor(
                    out=c0[:],
                    in0=kprep[:],
                    in1=qc_t[:, 0:DIM].unsqueeze(1).broadcast_to([P, K, DIM]),
                    op=mybir.AluOpType.add,
                )

                wf_tiles = []
                c_cur = c0
                for stage in range(2):
                    def _emit_sq(c_cur):
                        csq = ccpool.tile([P, K, DIM], F32, tag="csq")
                        nc.vector.tensor_tensor(
                            out=csq[:], in0=c_cur[:], in1=c_cur[:], op=mybir.AluOpType.mult
                        )
                        cc = ccpool.tile([P, K], F32, tag="ccb")
                        nc.vector.tensor_reduce(
                            out=cc[:], in_=csq[:], axis=mybir.AxisListType.X,
                            op=mybir.AluOpType.add,
                        )
                        # sq[q, n, k] = (|s|^2 + |C|^2) + sum_d (-2 s_d) C_d
                        base = sqpool.tile([P, NN, K], F32, tag="base")
                        nc.gpsimd.tensor_tensor(
                            out=base[:],
                            in0=sp4t[:, 3, :].unsqueeze(2).broadcast_to([P, NN, K]),
                            in1=cc[:].unsqueeze(1).broadcast_to([P, NN, K]),
                            op=mybir.AluOpType.add,
                        )
                        tx = sqpool.tile([P, NN, K], F32, tag="tx")
                        ty = sqpool.tile([P, NN, K], F32, tag="ty")
                        nc.vector.tensor_tensor(
                            out=tx[:],
                            in0=sp4t[:, 0, :].unsqueeze(2).broadcast_to([P, NN, K]),
                            in1=c_cur[:, :, 0].unsqueeze(1).broadcast_to([P, NN, K]),
                            op=mybir.AluOpType.mult,
                        )
                        _eng1 = nc.vector if os.environ.get("KGPS") == "dve" else nc.gpsimd
                        _eng1.tensor_tensor(
                            out=ty[:],
                            in0=sp4t[:, 1, :].unsqueeze(2).broadcast_to([P, NN, K]),
                            in1=c_cur[:, :, 1].unsqueeze(1).broadcast_to([P, NN, K]),
                            op=mybir.AluOpType.mult,
                        )
                        nc.vector.tensor_tensor(
                            out=tx[:], in0=tx[:], in1=ty[:], op=mybir.AluOpType.add
                        )
                        ty2 = sqpool.tile([P, NN, K], F32, tag="ty2")
                        _eng1.tensor_tensor(
                            out=ty2[:],
                            in0=sp4t[:, 2, :].unsqueeze(2).broadcast_to([P, NN, K]),
                            in1=c_cur[:, :, 2].unsqueeze(1).broadcast_to([P, NN, K]),
                            op=mybir.AluOpType.mult,
                        )
                        nc.vector.tensor_tensor(
                            out=tx[:], in0=tx[:], in1=ty2[:], op=mybir.AluOpType.add
                        )
                        sqt = sqpool.tile([P, NN, K], F32, tag="sqt")
                        nc.vector.tensor_tensor(
                            out=sqt[:], in0=tx[:], in1=base[:], op=mybir.AluOpType.add
                        )

                        # influence: w' = relu(2 - sqrt(sq))
                        dts = wdpool.tile([P, NN, K], BF16, tag="dts")
                        nc.scalar.activation(
                            out=dts[:], in_=sqt[:],
                            func=mybir.ActivationFunctionType.Sqrt, bias=eps_c[:],
                        )
                        wdense = wdpool.tile([P, NN * K], BF16, tag="wdense")
                        nc.vector.tensor_scalar(
                            out=wdense[:],
                            in0=dts[:].rearrange("p n k -> p (n k)"),
                            scalar1=2.0,
                            scalar2=2.0,
                            op0=mybir.AluOpType.min,
                            op1=mybir.AluOpType.subtract,
                        )

                        return wdense

                    if "sq" in skips:
                        wdense = wdpool.tile([P, NN * K], BF16, tag="wdense")
                        nc.vector.memset(wdense[:], 0.5)
                    else:
                        wdense = _emit_sq(c_cur)

                    if lvl < 3:
                        if stage == 0:
                            out_sb0 = opool.tile([P, F_OUT], F32, tag="outsb")
                            nc.vector.memset(out_sb0[:], 0.0)
                            nc.vector.tensor_copy(out=out_sb0[:, 0:240], in_=wdense[:, 0:240])
                            nc.sync.dma_start(out=out_d[t * P : (t + 1) * P, :], in_=out_sb0[:])
                        break
                    # scatter into block-diagonal tile via DRAM bounce
                    # (partition remap q-layout -> edge-slot layout)
                    wblk = wblks[stage * 3 + (t % 3)].ap()
                    if "scatter" not in skips:
                        bounce = drpool.tile([P, NN * K], BF16, tag="bounce")
                        nc.sync.dma_start(out=bounce[:], in_=wdense[:])
                        wsrc = bounce[:].rearrange(
                            "(g qq) (n k) -> qq n g k", qq=4, k=K
                        )
                        for qq in range(4):
                            nc.sync.dma_start(
                                out=wblk[32 * qq : 32 * (qq + 1), :, K * qq : K * (qq + 1)],
                                in_=wsrc[qq],
                            )

                    if lvl < 4:
                        if stage == 0:
                            out_sb0 = opool.tile([P, F_OUT], F32, tag="outsb")
                            nc.vector.memset(out_sb0[:], 0.0)
                            nc.sync.dma_start(out=out_d[t * P : (t + 1) * P, :], in_=out_sb0[:])
                        break
                    # neighbor contraction:  psum[f, (qq,k)] += nf^T . wblk
                    wf_sb = wfpool.tile([P, K, P], BF16, tag=f"wf{stage}")
                    for b in ([] if "mm" in skips else range(4)):
                        psb = pspool.tile([P, 8 * 4 * K], F32, tag="psb")
                        for g8 in range(8):
                            g = b * 8 + g8
                            nc.tensor.matmul(
                                out=psb[:, g8 * 60 : (g8 + 1) * 60],
                                lhsT=nf[:, g, :],
                                rhs=wblk[:, g, :],
                                start=True,
                                stop=True,
                            )
                        # drain bank -> wf_sb[:, k, 32b:32b+32]  (k-major)
                        drain_src = psb[:].rearrange(
                            "p (g qq k) -> p k g qq", g=8, qq=4
                        )
                        drain_dst = (
                            wf_sb[:, :, 32 * b : 32 * (b + 1)]
                            .rearrange("p k (g qq) -> p k g qq", qq=4)
                        )
                        nc.vector.tensor_copy(out=drain_dst, in_=drain_src)
                    wf_tiles.append(wf_sb)

                    if lvl < 5:
                        if stage == 0:
                            out_sb0 = opool.tile([P, F_OUT], F32, tag="outsb")
                            nc.vector.tensor_copy(out=out_sb0[:, 0:128], in_=wf_sb[:, 0, :])
                            nc.vector.memset(out_sb0[:, 128:], 0.0)
                            nc.sync.dma_start(out=out_d[t * P : (t + 1) * P, :], in_=out_sb0[:])
                        break
                    if stage == 0:
                        # offset projection: feat0[q, o] = sum_k wf0_k^T . dw_k
                        psA = ps2pool.tile([P, OFF_DIM], F32, tag="psA")
                        for k in range(K):
                            nc.tensor.matmul(
                                out=psA[:],
                                lhsT=wf_sb[:, k, :],
                                rhs=dwsb[:, k * OFF_DIM : (k + 1) * OFF_DIM],
                                start=(k == 0),
                                stop=(k == K - 1),
                            )
                        off_sb = ccpool.tile([P, OFF_DIM], F32, tag="off")
                        nc.vector.tensor_tensor(
                            out=off_sb[:], in0=psA[:], in1=brep[:],
                            op=mybir.AluOpType.add,
                        )
                        # C1 = C0 + offsets (k=0 offset is zero)
                        c1 = ccpool.tile([P, K, DIM], F32, tag="c1")
                        nc.vector.tensor_copy(out=c1[:, 0, :], in_=c0[:, 0, :])
                        nc.vector.tensor_tensor(
                            out=c1[:, 1:K, :],
                            in0=c0[:, 1:K, :],
                            in1=off_sb[:].rearrange("p (k d) -> p k d", d=DIM),
                            op=mybir.AluOpType.add,
                        )
                        c_cur = c1

                if lvl < 5:
                    continue
                # output projection: out[q, o] = sum_k wf1_k^T . W_k
                psO = ps2pool.tile([P, F_OUT], F32, tag="psO")
                wf1 = wf_tiles[1]
                for k in range(K):
                    nc.tensor.matmul(
                        out=psO[:],
                        lhsT=wf1[:, k, :],
                        rhs=wsb[:, k * F_OUT : (k + 1) * F_OUT],
                        start=(k == 0),
                        stop=(k == K - 1),
                    )
                out_sb = opool.tile([P, F_OUT], F32, tag="outsb")
                nc.vector.tensor_copy(out=out_sb[:], in_=psO[:])
                nc.sync.dma_start(out=out_d[t * P : (t + 1) * P, :], in_=out_sb[:])

    nc.compile()
    return nc


def _prep_shared(support_points, features, K_points, weight, deformable_weight, bias):
    f16 = features.astype(ml_dtypes.bfloat16)
    sp4 = np.empty((N_S, 4), dtype=np.float32)
    sp4[:, :3] = support_points
    sp4[:, 3] = (support_points.astype(np.float64) ** 2).sum(1)
    kprep = np.broadcast_to(
        K_points.reshape(1, K * DIM), (P, K * DIM)
    ).astype(np.float32).copy()
    dwsb = (
        deformable_weight.transpose(1, 0, 2).reshape(F_IN, K * OFF_DIM) * -0.5
    ).astype(ml_dtypes.bfloat16)
    wsb = (
        weight.transpose(1, 0, 2).reshape(F_IN, K * F_OUT) * -0.5
    ).astype(ml_dtypes.bfloat16)
    brep = np.broadcast_to(bias.reshape(1, OFF_DIM), (P, OFF_DIM)).astype(
        np.float32
    ).copy()
    return f16, sp4, kprep, dwsb, wsb, brep


def _prep_core(query_points, neighbors, qpc, f16, sp4):
    """Shard-local tensors: pregathered neighbor features (edge-slot layout)
    and support coords (query layout), per the all-gather-neighbor-features
    sharding strategy."""
    T = qpc // P
    nbr = neighbors.astype(np.int64).reshape(T, P, NN)
    p = np.arange(P)
    g = np.arange(NN)
    # edge-slot permutation: ie[t, p, g] = nbr[t, 4g + p//32, p%32]
    ie = nbr[:, (4 * g[None, :] + p[:, None] // 32), (p[:, None] % 32)]
    nfg = np.asarray(f16)[ie]          # [T, P, NN, F_IN] bf16
    spg = sp4[nbr]                     # [T, P, NN, 4] f32
    qc = np.zeros((T, P, 4), dtype=np.float32)
    qc[:, :, :3] = query_points.reshape(T, P, DIM)
    return nfg, spg, qc


def prepare(inputs):
    """Build the Bass program + per-core input maps from full inputs."""
    query_points = np.asarray(inputs["query_points"], dtype=np.float32)
    support_points = np.asarray(inputs["support_points"], dtype=np.float32)
    neighbors = np.asarray(inputs["neighbors"])
    features = np.asarray(inputs["features"], dtype=np.float32)
    K_points = np.asarray(inputs["K_points"], dtype=np.float32)
    weight = np.asarray(inputs["weight"], dtype=np.float32)
    deformable_weight = np.asarray(inputs["deformable_weight"], dtype=np.float32)
    bias = np.asarray(inputs["bias"], dtype=np.float32)

    qpc = N_Q // N_CORES
    f16, sp4, kprep, dwsb, wsb, brep = _prep_shared(
        support_points, features, K_points, weight, deformable_weight, bias)

    in_maps = []
    for c in range(N_CORES):
        sl = slice(c * qpc, (c + 1) * qpc)
        nfg, spg, qc = _prep_core(query_points[sl], neighbors[sl], qpc, f16, sp4)
        in_maps.append({
            "nfg": np.asarray(nfg), "spg": spg, "qc": qc,
            "kprep": kprep, "dwsb": np.asarray(dwsb), "wsb": np.asarray(wsb),
            "brep": brep,
        })

    nc = build_nc(qpc)
    return nc, in_maps


def collect(res):
    out = np.concatenate([res.results[c]["out"] for c in range(N_CORES)], axis=0)
    return out.astype(np.float32)


def kernel(**inputs):
    from concourse.bass_utils import run_bass_kernel_spmd

    nc, in_maps = prepare(inputs)
    res = run_bass_kernel_spmd(nc, in_maps, core_ids=list(range(N_CORES)))
    return collect(res)



# revision 6
# speedup vs baseline: 1.4825x; 1.4825x over previous
"""Deformable KPConv layer on 8 Trainium2 NeuronCores (Bass/Tile).

Strategy (data-parallel over the 16384 query points, 2048/core):
  - features are pre-cast to bf16 host-side and gathered per-edge from HBM
    into an "edge-slot" layout [(4 queries x 32 neighbors) partitions,
    group, 128 feat]; support coords (+|s|^2) in query-partition layout.
  - squared distances to (possibly deformed) kernel points are computed in
    k-major [P, K, NN] layout as |s|^2 + |C|^2 - 2 s.C with C = q + kp
    (+ offset); all broadcast operands have <=4B innermost strides.
    Influence w' = relu(2 - d) (the 1/2 is folded into the conv weights
    host-side). The chain is split DVE/GpSimd; PSUM drains run on ScalarE.
  - the neighbor contraction runs on TensorE as block-diagonal matmuls:
    w' is scattered into a zero-initialized block-diagonal SBUF tile by
    4 strided SBUF->SBUF DMAs (via a DRAM bounce), then
    psum[f,(q,k)] = nf^T @ wblk.
  - the (k,f)->42 offset projection and (k,f)->256 output projection are
    PSUM-accumulated matmuls producing query-partition outputs directly.
  - the emission order is software-pipelined: stage-0 of tile t+1 is
    emitted before stage-1 of tile t so the in-order PE queue never
    stalls on the offset round-trip.
"""

import sys

sys.path.insert(0, "/opt/trn_rl_repo")

import numpy as np
import ml_dtypes

import concourse.bass as bass
import concourse.tile as tile
from concourse import bacc, mybir

N_Q = 16384
N_S = 16384
NN = 32
F_IN = 128
F_OUT = 256
K = 15
DIM = 3
OFF_DIM = DIM * (K - 1)  # 42
EXTENT = 2.0
N_CORES = 8
P = 128

BF16 = mybir.dt.bfloat16
F32 = mybir.dt.float32

AF = mybir.ActivationFunctionType
ALU = mybir.AluOpType


def build_nc(qpc: int):
    """Build the single-core SPMD Bass program for qpc queries per core."""
    T = qpc // P  # query tiles per core
    NG = P // 4  # 32 groups of 4 queries per tile

    nc = bacc.Bacc("TRN2", target_bir_lowering=False)

    nfg_d = nc.dram_tensor("nfg", [T, P, NN, F_IN], BF16, kind="ExternalInput")
    spg_d = nc.dram_tensor("spg", [T, P, NN, 4], F32, kind="ExternalInput")
    qc_d = nc.dram_tensor("qc", [T, P, 4], F32, kind="ExternalInput")
    kprep_d = nc.dram_tensor("kprep", [P, DIM * K], F32, kind="ExternalInput")
    dwsb_d = nc.dram_tensor("dwsb", [P, K * OFF_DIM], BF16, kind="ExternalInput")
    wsb_d = nc.dram_tensor("wsb", [P, K * F_OUT], BF16, kind="ExternalInput")
    brep_d = nc.dram_tensor("brep", [P, OFF_DIM], F32, kind="ExternalInput")
    out_d = nc.dram_tensor("out", [qpc, F_OUT], F32, kind="ExternalOutput")

    with tile.TileContext(nc) as tc:
        with (
            tc.tile_pool(name="const", bufs=1) as cpool,
            tc.tile_pool(name="idx", bufs=3) as ipool,
            tc.tile_pool(name="nf", bufs=4) as nfpool,
            tc.tile_pool(name="sp", bufs=4) as sppool,
            tc.tile_pool(name="sq", bufs=3) as sqpool,
            tc.tile_pool(name="wd", bufs=3) as wdpool,
            tc.tile_pool(name="wf", bufs=2) as wfpool,
            tc.tile_pool(name="cc", bufs=3) as ccpool,
            tc.tile_pool(name="outp", bufs=2) as opool,
            tc.tile_pool(name="dram", bufs=4, space="DRAM") as drpool,
            tc.tile_pool(name="ps", bufs=4, space="PSUM") as pspool,
            tc.tile_pool(name="ps2", bufs=2, space="PSUM") as ps2pool,
        ):
            # --- constants, loaded once ---
            kprep = cpool.tile([P, DIM, K], F32, tag="kprep")
            nc.sync.dma_start(
                out=kprep[:], in_=kprep_d[:].rearrange("p (d k) -> p d k", k=K)
            )
            dwsb = cpool.tile([P, K * OFF_DIM], BF16, tag="dwsb")
            nc.sync.dma_start(out=dwsb[:], in_=dwsb_d[:])
            wsb = cpool.tile([P, K * F_OUT], BF16, tag="wsb")
            nc.sync.dma_start(out=wsb[:], in_=wsb_d[:])
            brep = cpool.tile([P, OFF_DIM], F32, tag="brep")
            nc.sync.dma_start(out=brep[:], in_=brep_d[:])
            eps_c = cpool.tile([P, 1], F32, tag="eps")
            nc.vector.memset(eps_c[:], 1e-5)

            # persistent block-diagonal tiles (zeros off-diagonal, never touched
            # there again: the scatter DMAs only overwrite the diagonal blocks)
            wblks = []
            for i in range(6):
                wb = nc.alloc_sbuf_tensor(f"wblk{i}", [P, NG, 4 * K], BF16)
                nc.gpsimd.memset(wb.ap(), 0.0)
                wblks.append(wb)

            def emit_sq_w(sp4t, cT, wblk, t):
                """Distance + influence for one stage; scatter w' into wblk."""
                # |C|^2 per kernel point: csq = cT*cT, cc = sum over d
                csq = ccpool.tile([P, DIM, K], F32, tag="csq")
                nc.vector.tensor_tensor(
                    out=csq[:], in0=cT[:], in1=cT[:], op=ALU.mult
                )
                ccb = ccpool.tile([P, K], F32, tag="ccb")
                nc.vector.tensor_tensor(
                    out=ccb[:], in0=csq[:, 0, :], in1=csq[:, 1, :], op=ALU.add
                )
                nc.vector.tensor_tensor(
                    out=ccb[:], in0=ccb[:], in1=csq[:, 2, :], op=ALU.add
                )
                # base[k, n] = |s|^2 + |C|^2   (gpsimd)
                base = sqpool.tile([P, K, NN], F32, tag="base")
                nc.gpsimd.tensor_tensor(
                    out=base[:],
                    in0=sp4t[:, 3, :].unsqueeze(1).broadcast_to([P, K, NN]),
                    in1=ccb[:].unsqueeze(2).broadcast_to([P, K, NN]),
                    op=ALU.add,
                )
                # cross terms: (-2 s_d) * C_d ; d=0 on DVE, d=1,2 on gpsimd
                tx = sqpool.tile([P, K, NN], F32, tag="tx")
                nc.vector.tensor_tensor(
                    out=tx[:],
                    in0=sp4t[:, 0, :].unsqueeze(1).broadcast_to([P, K, NN]),
                    in1=cT[:, 0, :].unsqueeze(2).broadcast_to([P, K, NN]),
                    op=ALU.mult,
                )
                ty = sqpool.tile([P, K, NN], F32, tag="ty")
                nc.gpsimd.tensor_tensor(
                    out=ty[:],
                    in0=sp4t[:, 1, :].unsqueeze(1).broadcast_to([P, K, NN]),
                    in1=cT[:, 1, :].unsqueeze(2).broadcast_to([P, K, NN]),
                    op=ALU.mult,
                )
                ty2 = sqpool.tile([P, K, NN], F32, tag="ty2")
                nc.gpsimd.tensor_tensor(
                    out=ty2[:],
                    in0=sp4t[:, 2, :].unsqueeze(1).broadcast_to([P, K, NN]),
                    in1=cT[:, 2, :].unsqueeze(2).broadcast_to([P, K, NN]),
                    op=ALU.mult,
                )
                nc.vector.tensor_tensor(
                    out=tx[:], in0=tx[:], in1=base[:], op=ALU.add
                )
                nc.vector.tensor_tensor(
                    out=tx[:], in0=tx[:], in1=ty[:], op=ALU.add
                )
                sqt = sqpool.tile([P, K, NN], F32, tag="sqt")
                nc.vector.tensor_tensor(
                    out=sqt[:], in0=tx[:], in1=ty2[:], op=ALU.add
                )

                # influence: w' = relu(2 - sqrt(sq)); the tensor_scalar
                # writes transposed back to (n, k) order so the scatter DMA
                # below keeps k contiguous on both sides.
                dts = wdpool.tile([P, K, NN], BF16, tag="dts")
                nc.scalar.activation(
                    out=dts[:], in_=sqt[:], func=AF.Sqrt, bias=eps_c[:]
                )
                wdense = wdpool.tile([P, NN * K], BF16, tag="wdense")
                nc.vector.tensor_scalar(
                    out=wdense[:].rearrange("p (n k) -> p k n", k=K),
                    in0=dts[:],
                    scalar1=2.0,
                    scalar2=2.0,
                    op0=ALU.min,
                    op1=ALU.subtract,
                )

                # scatter into block-diagonal tile via DRAM bounce
                # (partition remap q-layout -> edge-slot layout)
                bounce = drpool.tile([P, NN * K], BF16, tag="bounce")
                nc.sync.dma_start(out=bounce[:], in_=wdense[:])
                wsrc = bounce[:].rearrange("(g qq) (n k) -> qq n g k", qq=4, k=K)
                for qq in range(4):
                    nc.sync.dma_start(
                        out=wblk[32 * qq : 32 * (qq + 1), :, K * qq : K * (qq + 1)],
                        in_=wsrc[qq],
                    )

            def emit_contract(nf, wblk, stage):
                """psum[f, (qq,k)] += nf^T . wblk, drained (ScalarE) k-major."""
                wf_sb = wfpool.tile([P, K, P], BF16, tag=f"wf{stage}")
                for b in range(4):
                    psb = pspool.tile([P, 8 * 4 * K], F32, tag="psb")
                    for g8 in range(8):
                        g = b * 8 + g8
                        nc.tensor.matmul(
                            out=psb[:, g8 * 60 : (g8 + 1) * 60],
                            lhsT=nf[:, g, :],
                            rhs=wblk[:, g, :],
                            start=True,
                            stop=True,
                        )
                    drain_src = psb[:].rearrange("p (g qq k) -> p k g qq", g=8, qq=4)
                    drain_dst = (
                        wf_sb[:, :, 32 * b : 32 * (b + 1)]
                        .rearrange("p k (g qq) -> p k g qq", qq=4)
                    )
                    nc.vector.tensor_copy(out=drain_dst, in_=drain_src)
                return wf_sb

            def emit_stage0(t):
                qc_t = ipool.tile([P, 4], F32, tag="qc")
                nc.sync.dma_start(out=qc_t[:], in_=qc_d[t])
                nf = nfpool.tile([P, NN, F_IN], BF16, tag="nf")
                nc.sync.dma_start(out=nf[:], in_=nfg_d[t])
                sp = sppool.tile([P, NN, 4], F32, tag="sp")
                nc.sync.dma_start(out=sp[:], in_=spg_d[t])

                # s' = -2*s, stored d-major [P, 4, NN]; row 3 = +|s|^2
                sp4t = sppool.tile([P, 4, NN], F32, tag="sp4t")
                nc.vector.tensor_scalar(
                    out=sp4t[:, 0:DIM, :],
                    in0=sp[:].transpose([0, 2, 1])[:, 0:DIM, :],
                    scalar1=-2.0,
                    scalar2=None,
                    op0=ALU.mult,
                )
                nc.vector.tensor_scalar(
                    out=sp4t[:, 3, :],
                    in0=sp[:, :, 3],
                    scalar1=1.0,
                    scalar2=None,
                    op0=ALU.mult,
                )

                # C0[d, k] = q_d + kp[d, k]
                c0 = ccpool.tile([P, DIM, K], F32, tag="c0")
                nc.vector.tensor_tensor(
                    out=c0[:],
                    in0=kprep[:],
                    in1=qc_t[:, 0:DIM].unsqueeze(2).broadcast_to([P, DIM, K]),
                    op=ALU.add,
                )

                wblk = wblks[t % 3].ap()
                emit_sq_w(sp4t, c0, wblk, t)
                wf0 = emit_contract(nf, wblk, 0)

                # offset projection: feat0[q, o] = sum_k wf0_k^T . dw_k
                psA = ps2pool.tile([P, OFF_DIM], F32, tag="psA")
                for k in range(K):
                    nc.tensor.matmul(
                        out=psA[:],
                        lhsT=wf0[:, k, :],
                        rhs=dwsb[:, k * OFF_DIM : (k + 1) * OFF_DIM],
                        start=(k == 0),
                        stop=(k == K - 1),
                    )
                off_sb = ccpool.tile([P, OFF_DIM], F32, tag="off")
                nc.vector.tensor_tensor(
                    out=off_sb[:], in0=psA[:], in1=brep[:], op=ALU.add
                )
                # C1 = C0 + offsets (k=0 offset is zero)
                c1 = ccpool.tile([P, DIM, K], F32, tag="c1")
                nc.vector.tensor_copy(out=c1[:, :, 0], in_=c0[:, :, 0])
                nc.vector.tensor_tensor(
                    out=c1[:, :, 1:K],
                    in0=c0[:, :, 1:K],
                    in1=off_sb[:].rearrange("p (k d) -> p d k", d=DIM),
                    op=ALU.add,
                )
                return nf, sp4t, c1

            def emit_stage1(t, nf, sp4t, c1):
                wblk = wblks[3 + t % 3].ap()
                emit_sq_w(sp4t, c1, wblk, t)
                wf1 = emit_contract(nf, wblk, 1)

                psO = ps2pool.tile([P, F_OUT], F32, tag="psO")
                for k in range(K):
                    nc.tensor.matmul(
                        out=psO[:],
                        lhsT=wf1[:, k, :],
                        rhs=wsb[:, k * F_OUT : (k + 1) * F_OUT],
                        start=(k == 0),
                        stop=(k == K - 1),
                    )
                out_sb = opool.tile([P, F_OUT], F32, tag="outsb")
                nc.vector.tensor_copy(out=out_sb[:], in_=psO[:])
                nc.sync.dma_start(out=out_d[t * P : (t + 1) * P, :], in_=out_sb[:])

            # software-pipelined emission: stage-0 of tile t+1 goes before
            # stage-1 of tile t so the PE queue never stalls on the offset
            # round-trip of the current tile.
            prev = None
            for t in range(T):
                cur = (t, *emit_stage0(t))
                if prev is not None:
                    emit_stage1(*prev)
                prev = cur
            emit_stage1(*prev)

    nc.compile()
    return nc


def _prep_shared(support_points, features, K_points, weight, deformable_weight, bias):
    f16 = features.astype(ml_dtypes.bfloat16)
    sp4 = np.empty((N_S, 4), dtype=np.float32)
    sp4[:, :3] = support_points
    sp4[:, 3] = (support_points.astype(np.float64) ** 2).sum(1)
    kprep = np.broadcast_to(
        K_points.T.reshape(1, DIM * K), (P, DIM * K)
    ).astype(np.float32).copy()
    dwsb = (
        deformable_weight.transpose(1, 0, 2).reshape(F_IN, K * OFF_DIM) * -0.5
    ).astype(ml_dtypes.bfloat16)
    wsb = (
        weight.transpose(1, 0, 2).reshape(F_IN, K * F_OUT) * -0.5
    ).astype(ml_dtypes.bfloat16)
    brep = np.broadcast_to(bias.reshape(1, OFF_DIM), (P, OFF_DIM)).astype(
        np.float32
    ).copy()
    return f16, sp4, kprep, dwsb, wsb, brep


def _prep_core(query_points, neighbors, qpc, f16, sp4):
    """Shard-local tensors: pregathered neighbor features (edge-slot layout)
    and support coords (query layout), per the all-gather-neighbor-features
    sharding strategy."""
    T = qpc // P
    nbr = neighbors.astype(np.int64).reshape(T, P, NN)
    p = np.arange(P)
    g = np.arange(NN)
    # edge-slot permutation: ie[t, p, g] = nbr[t, 4g + p//32, p%32]
    ie = nbr[:, (4 * g[None, :] + p[:, None] // 32), (p[:, None] % 32)]
    nfg = np.asarray(f16)[ie]          # [T, P, NN, F_IN] bf16
    spg = sp4[nbr]                     # [T, P, NN, 4] f32
    qc = np.zeros((T, P, 4), dtype=np.float32)
    qc[:, :, :3] = query_points.reshape(T, P, DIM)
    return nfg, spg, qc


def prepare(inputs):
    """Build the Bass program + per-core input maps from full inputs."""
    query_points = np.asarray(inputs["query_points"], dtype=np.float32)
    support_points = np.asarray(inputs["support_points"], dtype=np.float32)
    neighbors = np.asarray(inputs["neighbors"])
    features = np.asarray(inputs["features"], dtype=np.float32)
    K_points = np.asarray(inputs["K_points"], dtype=np.float32)
    weight = np.asarray(inputs["weight"], dtype=np.float32)
    deformable_weight = np.asarray(inputs["deformable_weight"], dtype=np.float32)
    bias = np.asarray(inputs["bias"], dtype=np.float32)

    qpc = N_Q // N_CORES
    f16, sp4, kprep, dwsb, wsb, brep = _prep_shared(
        support_points, features, K_points, weight, deformable_weight, bias)

    in_maps = []
    for c in range(N_CORES):
        sl = slice(c * qpc, (c + 1) * qpc)
        nfg, spg, qc = _prep_core(query_points[sl], neighbors[sl], qpc, f16, sp4)
        in_maps.append({
            "nfg": np.asarray(nfg), "spg": spg, "qc": qc,
            "kprep": kprep, "dwsb": np.asarray(dwsb), "wsb": np.asarray(wsb),
            "brep": brep,
        })

    nc = build_nc(qpc)
    return nc, in_maps


def collect(res):
    out = np.concatenate([res.results[c]["out"] for c in range(N_CORES)], axis=0)
    return out.astype(np.float32)


def kernel(**inputs):
    from concourse.bass_utils import run_bass_kernel_spmd

    nc, in_maps = prepare(inputs)
    res = run_bass_kernel_spmd(nc, in_maps, core_ids=list(range(N_CORES)))
    return collect(res)


# revision 7
# speedup vs baseline: 1.4871x; 1.0031x over previous
"""Deformable KPConv layer on 8 Trainium2 NeuronCores (Bass/Tile).

Strategy (data-parallel over the 16384 query points, 2048/core):
  - features are pre-cast to bf16 host-side and gathered per-edge from HBM
    into an "edge-slot" layout [(4 queries x 32 neighbors) partitions,
    group, 128 feat]; support coords (+|s|^2) in query-partition layout.
  - squared distances to (possibly deformed) kernel points are computed in
    k-major [P, K, NN] layout as |s|^2 + |C|^2 - 2 s.C with C = q + kp
    (+ offset); all broadcast operands have <=4B innermost strides.
    Influence w' = relu(2 - d) (the 1/2 is folded into the conv weights
    host-side). The chain is split DVE/GpSimd; PSUM drains run on ScalarE.
  - the neighbor contraction runs on TensorE as block-diagonal matmuls:
    w' is scattered into a zero-initialized block-diagonal SBUF tile by
    4 strided SBUF->SBUF DMAs (via a DRAM bounce), then
    psum[f,(q,k)] = nf^T @ wblk.
  - the (k,f)->42 offset projection and (k,f)->256 output projection are
    PSUM-accumulated matmuls producing query-partition outputs directly.
  - the emission order is software-pipelined: stage-0 of tile t+1 is
    emitted before stage-1 of tile t so the in-order PE queue never
    stalls on the offset round-trip.
"""

import sys

sys.path.insert(0, "/opt/trn_rl_repo")

import numpy as np
import ml_dtypes

import concourse.bass as bass
import concourse.tile as tile
from concourse import bacc, mybir

N_Q = 16384
N_S = 16384
NN = 32
F_IN = 128
F_OUT = 256
K = 15
DIM = 3
OFF_DIM = DIM * (K - 1)  # 42
EXTENT = 2.0
N_CORES = 8
P = 128

BF16 = mybir.dt.bfloat16
F32 = mybir.dt.float32

AF = mybir.ActivationFunctionType
ALU = mybir.AluOpType


def build_nc(qpc: int):
    """Build the single-core SPMD Bass program for qpc queries per core."""
    T = qpc // P  # query tiles per core
    NG = P // 4  # 32 groups of 4 queries per tile

    nc = bacc.Bacc("TRN2", target_bir_lowering=False)

    nfg_d = nc.dram_tensor("nfg", [T, P, NN, F_IN], BF16, kind="ExternalInput")
    spg_d = nc.dram_tensor("spg", [T, P, NN, 4], F32, kind="ExternalInput")
    qc_d = nc.dram_tensor("qc", [T, P, 4], F32, kind="ExternalInput")
    kprep_d = nc.dram_tensor("kprep", [P, DIM * K], F32, kind="ExternalInput")
    dwsb_d = nc.dram_tensor("dwsb", [P, K * OFF_DIM], BF16, kind="ExternalInput")
    wsb_d = nc.dram_tensor("wsb", [P, K * F_OUT], BF16, kind="ExternalInput")
    brep_d = nc.dram_tensor("brep", [P, OFF_DIM], F32, kind="ExternalInput")
    out_d = nc.dram_tensor("out", [qpc, F_OUT], F32, kind="ExternalOutput")

    with tile.TileContext(nc) as tc:
        with (
            tc.tile_pool(name="const", bufs=1) as cpool,
            tc.tile_pool(name="idx", bufs=3) as ipool,
            tc.tile_pool(name="nf", bufs=4) as nfpool,
            tc.tile_pool(name="sp", bufs=4) as sppool,
            tc.tile_pool(name="sq", bufs=3) as sqpool,
            tc.tile_pool(name="wd", bufs=3) as wdpool,
            tc.tile_pool(name="wf", bufs=2) as wfpool,
            tc.tile_pool(name="cc", bufs=3) as ccpool,
            tc.tile_pool(name="outp", bufs=2) as opool,
            tc.tile_pool(name="dram", bufs=4, space="DRAM") as drpool,
            tc.tile_pool(name="ps", bufs=4, space="PSUM") as pspool,
            tc.tile_pool(name="ps2", bufs=2, space="PSUM") as ps2pool,
        ):
            # --- constants, loaded once ---
            kprep = cpool.tile([P, DIM, K], F32, tag="kprep")
            nc.sync.dma_start(
                out=kprep[:], in_=kprep_d[:].rearrange("p (d k) -> p d k", k=K)
            )
            dwsb = cpool.tile([P, K * OFF_DIM], BF16, tag="dwsb")
            nc.sync.dma_start(out=dwsb[:], in_=dwsb_d[:])
            wsb = cpool.tile([P, K * F_OUT], BF16, tag="wsb")
            nc.sync.dma_start(out=wsb[:], in_=wsb_d[:])
            brep = cpool.tile([P, OFF_DIM], F32, tag="brep")
            nc.sync.dma_start(out=brep[:], in_=brep_d[:])
            eps_c = cpool.tile([P, 1], F32, tag="eps")
            nc.vector.memset(eps_c[:], 1e-5)

            # persistent block-diagonal tiles (zeros off-diagonal, never touched
            # there again: the scatter DMAs only overwrite the diagonal blocks)
            wblks = []
            for i in range(6):
                wb = nc.alloc_sbuf_tensor(f"wblk{i}", [P, NG, 4 * K], BF16)
                nc.gpsimd.memset(wb.ap(), 0.0)
                wblks.append(wb)

            def emit_sq_w(sp4t, cT, wblk, t):
                """Distance + influence for one stage; scatter w' into wblk."""
                # |C|^2 per kernel point: csq = cT*cT, cc = sum over d
                csq = ccpool.tile([P, DIM, K], F32, tag="csq")
                nc.vector.tensor_tensor(
                    out=csq[:], in0=cT[:], in1=cT[:], op=ALU.mult
                )
                ccb = ccpool.tile([P, K], F32, tag="ccb")
                nc.vector.tensor_tensor(
                    out=ccb[:], in0=csq[:, 0, :], in1=csq[:, 1, :], op=ALU.add
                )
                nc.vector.tensor_tensor(
                    out=ccb[:], in0=ccb[:], in1=csq[:, 2, :], op=ALU.add
                )
                # base[k, n] = |s|^2 + |C|^2   (gpsimd)
                base = sqpool.tile([P, K, NN], F32, tag="base")
                nc.gpsimd.tensor_tensor(
                    out=base[:],
                    in0=sp4t[:, 3, :].unsqueeze(1).broadcast_to([P, K, NN]),
                    in1=ccb[:].unsqueeze(2).broadcast_to([P, K, NN]),
                    op=ALU.add,
                )
                # cross terms: (-2 s_d) * C_d ; d=0 on DVE, d=1,2 on gpsimd
                tx = sqpool.tile([P, K, NN], F32, tag="tx")
                nc.vector.tensor_tensor(
                    out=tx[:],
                    in0=sp4t[:, 0, :].unsqueeze(1).broadcast_to([P, K, NN]),
                    in1=cT[:, 0, :].unsqueeze(2).broadcast_to([P, K, NN]),
                    op=ALU.mult,
                )
                ty = sqpool.tile([P, K, NN], F32, tag="ty")
                nc.gpsimd.tensor_tensor(
                    out=ty[:],
                    in0=sp4t[:, 1, :].unsqueeze(1).broadcast_to([P, K, NN]),
                    in1=cT[:, 1, :].unsqueeze(2).broadcast_to([P, K, NN]),
                    op=ALU.mult,
                )
                ty2 = sqpool.tile([P, K, NN], F32, tag="ty2")
                nc.gpsimd.tensor_tensor(
                    out=ty2[:],
                    in0=sp4t[:, 2, :].unsqueeze(1).broadcast_to([P, K, NN]),
                    in1=cT[:, 2, :].unsqueeze(2).broadcast_to([P, K, NN]),
                    op=ALU.mult,
                )
                # cross-sum on gpsimd keeps its chain independent of DVE's
                tz = sqpool.tile([P, K, NN], F32, tag="tz")
                nc.gpsimd.tensor_tensor(
                    out=tz[:], in0=ty[:], in1=ty2[:], op=ALU.add
                )
                nc.vector.tensor_tensor(
                    out=tx[:], in0=tx[:], in1=base[:], op=ALU.add
                )
                sqt = sqpool.tile([P, K, NN], F32, tag="sqt")
                nc.vector.tensor_tensor(
                    out=sqt[:], in0=tx[:], in1=tz[:], op=ALU.add
                )

                # influence: w' = relu(2 - sqrt(sq)); the ScalarE sqrt writes
                # its output transposed back to (n, k) order so both the DVE
                # tensor_scalar and the scatter DMA below run dense.
                dts = wdpool.tile([P, NN, K], BF16, tag="dts")
                nc.scalar.activation(
                    out=dts[:].rearrange("p n k -> p k n"),
                    in_=sqt[:],
                    func=AF.Sqrt,
                    bias=eps_c[:],
                )
                wdense = wdpool.tile([P, NN * K], BF16, tag="wdense")
                nc.vector.tensor_scalar(
                    out=wdense[:],
                    in0=dts[:].rearrange("p n k -> p (n k)"),
                    scalar1=2.0,
                    scalar2=2.0,
                    op0=ALU.min,
                    op1=ALU.subtract,
                )

                # scatter into block-diagonal tile via DRAM bounce
                # (partition remap q-layout -> edge-slot layout)
                bounce = drpool.tile([P, NN * K], BF16, tag="bounce")
                nc.sync.dma_start(out=bounce[:], in_=wdense[:])
                wsrc = bounce[:].rearrange("(g qq) (n k) -> qq n g k", qq=4, k=K)
                for qq in range(4):
                    nc.sync.dma_start(
                        out=wblk[32 * qq : 32 * (qq + 1), :, K * qq : K * (qq + 1)],
                        in_=wsrc[qq],
                    )

            def emit_contract(nf, wblk, stage):
                """psum[f, (qq,k)] += nf^T . wblk, drained (ScalarE) k-major."""
                wf_sb = wfpool.tile([P, K, P], BF16, tag=f"wf{stage}")
                for b in range(4):
                    psb = pspool.tile([P, 8 * 4 * K], F32, tag="psb")
                    for g8 in range(8):
                        g = b * 8 + g8
                        nc.tensor.matmul(
                            out=psb[:, g8 * 60 : (g8 + 1) * 60],
                            lhsT=nf[:, g, :],
                            rhs=wblk[:, g, :],
                            start=True,
                            stop=True,
                        )
                    drain_src = psb[:].rearrange("p (g qq k) -> p k g qq", g=8, qq=4)
                    drain_dst = (
                        wf_sb[:, :, 32 * b : 32 * (b + 1)]
                        .rearrange("p k (g qq) -> p k g qq", qq=4)
                    )
                    nc.vector.tensor_copy(out=drain_dst, in_=drain_src)
                return wf_sb

            def emit_stage0(t):
                qc_t = ipool.tile([P, 4], F32, tag="qc")
                nc.sync.dma_start(out=qc_t[:], in_=qc_d[t])
                nf = nfpool.tile([P, NN, F_IN], BF16, tag="nf")
                nc.sync.dma_start(out=nf[:], in_=nfg_d[t])
                sp = sppool.tile([P, NN, 4], F32, tag="sp")
                nc.sync.dma_start(out=sp[:], in_=spg_d[t])

                # s' = -2*s, stored d-major [P, 4, NN]; row 3 = +|s|^2
                sp4t = sppool.tile([P, 4, NN], F32, tag="sp4t")
                nc.vector.tensor_scalar(
                    out=sp4t[:, 0:DIM, :],
                    in0=sp[:].transpose([0, 2, 1])[:, 0:DIM, :],
                    scalar1=-2.0,
                    scalar2=None,
                    op0=ALU.mult,
                )
                nc.vector.tensor_scalar(
                    out=sp4t[:, 3, :],
                    in0=sp[:, :, 3],
                    scalar1=1.0,
                    scalar2=None,
                    op0=ALU.mult,
                )

                # C0[d, k] = q_d + kp[d, k]
                c0 = ccpool.tile([P, DIM, K], F32, tag="c0")
                nc.vector.tensor_tensor(
                    out=c0[:],
                    in0=kprep[:],
                    in1=qc_t[:, 0:DIM].unsqueeze(2).broadcast_to([P, DIM, K]),
                    op=ALU.add,
                )

                wblk = wblks[t % 3].ap()
                emit_sq_w(sp4t, c0, wblk, t)
                wf0 = emit_contract(nf, wblk, 0)

                # offset projection: feat0[q, o] = sum_k wf0_k^T . dw_k
                psA = ps2pool.tile([P, OFF_DIM], F32, tag="psA")
                for k in range(K):
                    nc.tensor.matmul(
                        out=psA[:],
                        lhsT=wf0[:, k, :],
                        rhs=dwsb[:, k * OFF_DIM : (k + 1) * OFF_DIM],
                        start=(k == 0),
                        stop=(k == K - 1),
                    )
                off_sb = ccpool.tile([P, OFF_DIM], F32, tag="off")
                nc.vector.tensor_tensor(
                    out=off_sb[:], in0=psA[:], in1=brep[:], op=ALU.add
                )
                # C1 = C0 + offsets (k=0 offset is zero)
                c1 = ccpool.tile([P, DIM, K], F32, tag="c1")
                nc.vector.tensor_copy(out=c1[:, :, 0], in_=c0[:, :, 0])
                nc.vector.tensor_tensor(
                    out=c1[:, :, 1:K],
                    in0=c0[:, :, 1:K],
                    in1=off_sb[:].rearrange("p (k d) -> p d k", d=DIM),
                    op=ALU.add,
                )
                return nf, sp4t, c1

            def emit_stage1(t, nf, sp4t, c1):
                wblk = wblks[3 + t % 3].ap()
                emit_sq_w(sp4t, c1, wblk, t)
                wf1 = emit_contract(nf, wblk, 1)

                psO = ps2pool.tile([P, F_OUT], F32, tag="psO")
                for k in range(K):
                    nc.tensor.matmul(
                        out=psO[:],
                        lhsT=wf1[:, k, :],
                        rhs=wsb[:, k * F_OUT : (k + 1) * F_OUT],
                        start=(k == 0),
                        stop=(k == K - 1),
                    )
                out_sb = opool.tile([P, F_OUT], F32, tag="outsb")
                nc.vector.tensor_copy(out=out_sb[:], in_=psO[:])
                nc.sync.dma_start(out=out_d[t * P : (t + 1) * P, :], in_=out_sb[:])

            # software-pipelined emission: stage-0 of tile t+1 goes before
            # stage-1 of tile t so the PE queue never stalls on the offset
            # round-trip of the current tile.
            prev = None
            for t in range(T):
                cur = (t, *emit_stage0(t))
                if prev is not None:
                    emit_stage1(*prev)
                prev = cur
            emit_stage1(*prev)

    nc.compile()
    return nc


def _prep_shared(support_points, features, K_points, weight, deformable_weight, bias):
    f16 = features.astype(ml_dtypes.bfloat16)
    sp4 = np.empty((N_S, 4), dtype=np.float32)
    sp4[:, :3] = support_points
    sp4[:, 3] = (support_points.astype(np.float64) ** 2).sum(1)
    kprep = np.broadcast_to(
        K_points.T.reshape(1, DIM * K), (P, DIM * K)
    ).astype(np.float32).copy()
    dwsb = (
        deformable_weight.transpose(1, 0, 2).reshape(F_IN, K * OFF_DIM) * -0.5
    ).astype(ml_dtypes.bfloat16)
    wsb = (
        weight.transpose(1, 0, 2).reshape(F_IN, K * F_OUT) * -0.5
    ).astype(ml_dtypes.bfloat16)
    brep = np.broadcast_to(bias.reshape(1, OFF_DIM), (P, OFF_DIM)).astype(
        np.float32
    ).copy()
    return f16, sp4, kprep, dwsb, wsb, brep


def _prep_core(query_points, neighbors, qpc, f16, sp4):
    """Shard-local tensors: pregathered neighbor features (edge-slot layout)
    and support coords (query layout), per the all-gather-neighbor-features
    sharding strategy."""
    T = qpc // P
    nbr = neighbors.astype(np.int64).reshape(T, P, NN)
    p = np.arange(P)
    g = np.arange(NN)
    # edge-slot permutation: ie[t, p, g] = nbr[t, 4g + p//32, p%32]
    ie = nbr[:, (4 * g[None, :] + p[:, None] // 32), (p[:, None] % 32)]
    nfg = np.asarray(f16)[ie]          # [T, P, NN, F_IN] bf16
    spg = sp4[nbr]                     # [T, P, NN, 4] f32
    qc = np.zeros((T, P, 4), dtype=np.float32)
    qc[:, :, :3] = query_points.reshape(T, P, DIM)
    return nfg, spg, qc


def prepare(inputs):
    """Build the Bass program + per-core input maps from full inputs."""
    query_points = np.asarray(inputs["query_points"], dtype=np.float32)
    support_points = np.asarray(inputs["support_points"], dtype=np.float32)
    neighbors = np.asarray(inputs["neighbors"])
    features = np.asarray(inputs["features"], dtype=np.float32)
    K_points = np.asarray(inputs["K_points"], dtype=np.float32)
    weight = np.asarray(inputs["weight"], dtype=np.float32)
    deformable_weight = np.asarray(inputs["deformable_weight"], dtype=np.float32)
    bias = np.asarray(inputs["bias"], dtype=np.float32)

    qpc = N_Q // N_CORES
    f16, sp4, kprep, dwsb, wsb, brep = _prep_shared(
        support_points, features, K_points, weight, deformable_weight, bias)

    in_maps = []
    for c in range(N_CORES):
        sl = slice(c * qpc, (c + 1) * qpc)
        nfg, spg, qc = _prep_core(query_points[sl], neighbors[sl], qpc, f16, sp4)
        in_maps.append({
            "nfg": np.asarray(nfg), "spg": spg, "qc": qc,
            "kprep": kprep, "dwsb": np.asarray(dwsb), "wsb": np.asarray(wsb),
            "brep": brep,
        })

    nc = build_nc(qpc)
    return nc, in_maps


def collect(res):
    out = np.concatenate([res.results[c]["out"] for c in range(N_CORES)], axis=0)
    return out.astype(np.float32)


def kernel(**inputs):
    from concourse.bass_utils import run_bass_kernel_spmd

    nc, in_maps = prepare(inputs)
    res = run_bass_kernel_spmd(nc, in_maps, core_ids=list(range(N_CORES)))
    return collect(res)
